# revision 1
# baseline (speedup 1.0000x reference)
"""Variant B2: all-fp16 matmul with 2-way Dekker splits of features+weights.

fp16 has 11 mantissa bits, so h+l covers 22 bits and fp16*fp16 products are
exact in the fp32 PSUM accumulator; expected absmax ~1e-6 (fp32-envelope).

Per 512-token pack: token(p, g) = base + p*4 + g, 4 groups of 32 slots:
  slot g*32 + r      (r<6): h_r = bf16(comp_r)          x W row H_r=bf16(W_r)
  slot g*32 + 6 + r  (r<6): h_r (copy)                  x W row L_r=bf16(W_r-H_r)
  slot g*32 + 12 + r (r<6): l_r = bf16(comp_r - h_r)    x W row H_r
  slot g*32 + 18 + e (e<3): m_e (exact in bf16)         x W row Hb_e
  slot g*32 + 21 + e (e<3): m_e (copy)                  x W row Lb_e
  slots 24..31: zero
comp_r for r = 2*ei+k is feat[t,k]*(bus_type[t]==ei+1).
"""

import sys
from contextlib import ExitStack

import ml_dtypes
import numpy as np

sys.path.insert(0, "/opt/trn_rl_repo")

import concourse.bacc as bacc  # noqa: E402
import concourse.mybir as mybir  # noqa: E402
import concourse.tile as tile  # noqa: E402
from concourse.bass_utils import run_bass_kernel_spmd  # noqa: E402

FP = mybir.dt.float32
BF = mybir.dt.float16
D = 128
PACK = 512
SUPERS = [16384] * 7 + [11264]
N_CORES = 8
PER_CORE = sum(SUPERS)

_NC_CACHE = {}


def _body(ctx, tc, out, feat, btf, wbig, ident, n_tokens):
    nc = tc.nc
    assert n_tokens == sum(SUPERS)
    eq = mybir.AluOpType.is_equal
    mult = mybir.AluOpType.mult

    const_pool = ctx.enter_context(tc.tile_pool(name="const", bufs=1))
    wbig_sb = const_pool.tile([128, 512], BF)
    nc.sync.dma_start(wbig_sb[:], wbig)
    ident_sb = const_pool.tile([128, 128], BF)
    nc.sync.dma_start(ident_sb[:], ident)

    in_pool = ctx.enter_context(tc.tile_pool(name="inp", bufs=8))
    sc_pool = ctx.enter_context(tc.tile_pool(name="scr", bufs=2))
    tp_ps = ctx.enter_context(tc.tile_pool(name="tp_ps", bufs=2, space="PSUM"))
    pk_pool = ctx.enter_context(tc.tile_pool(name="pk", bufs=2))
    xsb_pool = ctx.enter_context(tc.tile_pool(name="xsb", bufs=6))
    mm_pool = ctx.enter_context(tc.tile_pool(name="mm", bufs=3, space="PSUM"))
    out_pool = ctx.enter_context(tc.tile_pool(name="outp", bufs=3))

    # prefetch every supertile's inputs up front (1.5 MB) so they never queue
    # behind the 2 MB output DMAs in the HWDGE ring
    in_tiles = []
    tbase = 0
    for si, ssz in enumerate(SUPERS):
        npk = ssz // PACK
        f01_v = feat[tbase:tbase + ssz, :].rearrange(
            "(p j) k -> p (j k)", p=128)  # [128, npk*8]
        bt_v = btf[tbase:tbase + ssz].rearrange("(p f) -> p f", p=128)
        f01T = in_pool.tile([128, 256], FP, tag="fc", name=f"f01T{si}")
        nc.sync.dma_start(f01T[:, :npk * 8], f01_v)
        btT = in_pool.tile([128, 128], FP, tag="btc", name=f"btT{si}")
        nc.sync.dma_start(btT[:, :npk * 4], bt_v)
        in_tiles.append((f01T, btT))
        tbase += ssz

    P_tiles = [None] * len(SUPERS)

    def build(si):
        # emit the DVE slot-build for supertile si (software-pipelined one
        # supertile ahead of the PE pack loop)
        npk = SUPERS[si] // PACK
        f01T, btT = in_tiles[si]
        P = pk_pool.tile([128, 4096], BF, name=f"P{si}")
        P_tiles[si] = P
        P4 = P.rearrange("p (a b c) -> p a b c", a=32, b=4, c=32)[:, :npk]
        btT3 = btT[:, :npk * 4].rearrange("p (a b) -> p a b", b=4)
        f0T3 = f01T[:, 0:npk * 8:2].rearrange("p (a b) -> p a b", b=4)
        f1T3 = f01T[:, 1:npk * 8:2].rearrange("p (a b) -> p a b", b=4)
        nc.vector.memset(P4[:, :, :, 24:32], 0.0)
        C = sc_pool.tile([128, 128], FP, name=f"C{si}")
        C3 = C.rearrange("p (a b) -> p a b", a=32, b=4)[:, :npk]
        for r in range(6):
            ei, k = divmod(r, 2)
            fT3 = f0T3 if k == 0 else f1T3
            nc.vector.scalar_tensor_tensor(C3[:], btT3[:], float(ei + 1),
                                           fT3[:], op0=eq, op1=mult)
            nc.vector.tensor_copy(P4[:, :, :, r], C3[:])  # h (cast to fp16)
            nc.vector.tensor_copy(P4[:, :, :, 6 + r], P4[:, :, :, r])
            nc.vector.tensor_sub(P4[:, :, :, 12 + r], C3[:],
                                 P4[:, :, :, r])  # l
        for e in (1, 2, 3):
            nc.vector.tensor_scalar(P4[:, :, :, 17 + e], btT3[:], float(e),
                                    None, op0=eq)
            nc.vector.tensor_copy(P4[:, :, :, 20 + e], P4[:, :, :, 17 + e])

    build(0)
    tbase = 0
    for si, ssz in enumerate(SUPERS):
        npk = ssz // PACK  # 32 or 22
        if si + 1 < len(SUPERS):
            build(si + 1)
        P = P_tiles[si]

        for blk_start in range(0, npk, 8):
            bpk = min(8, npk - blk_start)  # packs in this output block
            ob = out_pool.tile([128, 4096], FP)
            for q in range(0, bpk, 2):
                mm = mm_pool.tile([128, 1024], FP)
                for h in range(2):
                    pt = blk_start + q + h
                    xps = tp_ps.tile([128, 128], BF, tag="tp")
                    nc.tensor.transpose(xps[:], P[:, pt * 128:(pt + 1) * 128],
                                        ident_sb[:])
                    xsb = xsb_pool.tile([128, 128], BF)
                    nc.vector.tensor_copy(xsb[:], xps[:])
                    nc.tensor.matmul(mm[:, h * 512:(h + 1) * 512], xsb[:],
                                     wbig_sb[:], start=True, stop=True)
                nc.scalar.activation(ob[:, q * 512:(q + 2) * 512], mm[:],
                                     mybir.ActivationFunctionType.Tanh)
            out_blk = out[tbase + blk_start * PACK:
                          tbase + (blk_start + bpk) * PACK, :].rearrange(
                "(pk p g) d -> p pk (g d)", p=128, g=4)
            nc.sync.dma_start(out_blk,
                              ob[:, :bpk * 512].rearrange(
                                  "p (pk gd) -> p pk gd", pk=bpk))
        tbase += ssz


def build_nc(n_tokens=PER_CORE):
    key = n_tokens
    if key in _NC_CACHE:
        return _NC_CACHE[key]
    nc = bacc.Bacc("TRN2", target_bir_lowering=False, debug=False)
    feat = nc.dram_tensor("feat", [n_tokens, 2], FP, kind="ExternalInput").ap()
    btf = nc.dram_tensor("btf", [n_tokens], FP, kind="ExternalInput").ap()
    wbig = nc.dram_tensor("wbig", [128, 512], BF, kind="ExternalInput").ap()
    ident = nc.dram_tensor("ident", [128, 128], BF, kind="ExternalInput").ap()
    out = nc.dram_tensor("out", [n_tokens, D], FP, kind="ExternalOutput").ap()
    with tile.TileContext(nc) as tc:
        with ExitStack() as ctx:
            _body(ctx, tc, out, feat, btf, wbig, ident, n_tokens)
    nc.compile()
    _NC_CACHE[key] = nc
    return nc


def make_wbig(W_slack, b_slack, W_gen, b_gen, W_load, b_load):
    W_list = [np.asarray(w, np.float32) for w in (W_slack, W_gen, W_load)]
    b_list = [np.asarray(b, np.float32) for b in (b_slack, b_gen, b_load)]
    bf = np.float16
    WBig = np.zeros((128, 512), bf)
    for g in range(4):
        col = g * 128
        base = g * 32
        for r in range(6):
            ei, k = divmod(r, 2)
            W = W_list[ei][k]
            H = W.astype(bf)
            L = (W - H.astype(np.float32)).astype(bf)
            WBig[base + r, col:col + 128] = H
            WBig[base + 6 + r, col:col + 128] = L
            WBig[base + 12 + r, col:col + 128] = H
        for e in range(3):
            b = b_list[e]
            Hb = b.astype(bf)
            Lb = (b - Hb.astype(np.float32)).astype(bf)
            WBig[base + 18 + e, col:col + 128] = Hb
            WBig[base + 21 + e, col:col + 128] = Lb
    return WBig


def _permute_for_device(arr):
    s = arr.shape[1:]
    arr = arr.reshape(N_CORES, PER_CORE, *s)
    chunks = []
    off = 0
    for ssz in SUPERS:
        npk = ssz // PACK
        v = arr[:, off:off + ssz].reshape(N_CORES, npk, 128, 4, -1)
        chunks.append(np.ascontiguousarray(v.transpose(0, 2, 1, 3, 4))
                      .reshape(N_CORES, ssz, -1))
        off += ssz
    return np.concatenate(chunks, axis=1).reshape(N_CORES, PER_CORE, *s)


def kernel(feat, bus_type, W_slack, b_slack, W_gen, b_gen, W_load, b_load,
           **run_kwargs):
    feat = np.asarray(feat, np.float32)
    bt = np.asarray(bus_type)
    n = feat.shape[0]
    npad = N_CORES * PER_CORE
    assert n <= npad

    featp = np.zeros((npad, 2), np.float32)
    featp[:n] = feat
    btp = np.zeros((npad, 1), np.float32)
    btp[:n, 0] = bt.astype(np.float32)
    featd = _permute_for_device(featp)
    btd = _permute_for_device(btp)[:, :, 0]
    wbig = make_wbig(W_slack, b_slack, W_gen, b_gen, W_load, b_load)
    ident = np.eye(128, dtype=np.float16)

    nc = build_nc(PER_CORE)
    in_maps = [
        {"feat": featd[i], "btf": btd[i], "wbig": wbig, "ident": ident}
        for i in range(N_CORES)
    ]
    try:
        res = run_bass_kernel_spmd(nc, in_maps, list(range(N_CORES)),
                                   **run_kwargs)
    except Exception:
        # A previously-failed process can leave the NeuronCores wedged
        # (NRT_EXEC_UNIT_UNRECOVERABLE); a small probe op resets them.
        import time as _time

        import jax
        import jax.numpy as jnp

        for _ in range(3):
            try:
                float(jnp.sum(jnp.ones((8, 8))))
                break
            except Exception:
                _time.sleep(5)
        res = run_bass_kernel_spmd(nc, in_maps, list(range(N_CORES)),
                                   **run_kwargs)
    out = np.concatenate([res.results[i]["out"] for i in range(N_CORES)],
                         axis=0)
    kernel.last_result = res
    return out[:n]



# revision 2
# speedup vs baseline: 1.6271x; 1.6271x over previous
"""V3: fp16-output MoE routing kernel.

The correctness gate is rel_err < 2e-2; plain fp16 arithmetic gives ~2e-3,
so drop the Dekker splits of the old kernel and halve HBM output traffic by
writing fp16 (host casts back to fp32). Engine budget per core (125,952
tokens): ACT tanh ~116us (bottleneck), DMA ~100us, PE ~60us, DVE ~62us.

Data flow per 512-token pack:
- P[p, 32g+s] fp16 slot matrix, 9 slots used per 32-slot group:
    s=2*ei+k (<6): (bus_type==ei+1)*feat_k   s=6+e: (bus_type==e+1)
- DVE stream-transpose (32x32 blocks): X32[32A+i, 32B+j] = P[32A+j, 32B+i]
- matmul(out, lhsT=X32, rhs=Wbig) with Wbig block-diagonal over A-blocks:
    out[32B+j, A*128+d] = z(token(p=32A+j, g=B), d)    (one PSUM bank/pack)
- ACT tanh over 4 packs at a time (FD=2048, 4 PSUM banks), fp16 to SBUF
- output dumped linearly [128, npk*512]; host permutation is chosen so the
  per-partition HBM lines are fully contiguous and the host-side unpack is a
  pure reshape.

Host input permute: orig token q*(npk*4) + pt*4 + A  ->  device slot
(p = 32A + (q%32), pack pt, g = q//32), so device out column order
(p', pt, A, d) reads back as ascending original token index.
"""

import sys
from contextlib import ExitStack

import numpy as np

sys.path.insert(0, "/opt/trn_rl_repo")

import concourse.bacc as bacc  # noqa: E402
import concourse.mybir as mybir  # noqa: E402
import concourse.tile as tile  # noqa: E402
from concourse.bass_utils import run_bass_kernel_spmd  # noqa: E402

FP = mybir.dt.float32
F16 = mybir.dt.float16
D = 128
PACK = 512
SUPERS = [16384] * 7 + [11264]
NPKS = [s // PACK for s in SUPERS]
N_CORES = 8
PER_CORE = sum(SUPERS)

_NC_CACHE = {}


def _body(ctx, tc, out, featc, btc, wbig):
    nc = tc.nc
    eq = mybir.AluOpType.is_equal
    mult = mybir.AluOpType.mult

    const_pool = ctx.enter_context(tc.tile_pool(name="const", bufs=1))
    wbig_sb = const_pool.tile([128, 512], F16)
    nc.sync.dma_start(wbig_sb[:], wbig)
    # Persistent double-buffered slot matrices; slots 9..31 are zeroed once
    # and never written again (their Wbig rows are zero anyway).
    P_tiles = [const_pool.tile([128, 32 * 128], F16, name=f"Pbuf{i}")
               for i in range(2)]
    for Pt in P_tiles:
        nc.vector.memset(Pt[:], 0.0)

    in_pool = ctx.enter_context(tc.tile_pool(name="inp", bufs=8))
    x_pool = ctx.enter_context(tc.tile_pool(name="x32", bufs=8))
    mm_pool = ctx.enter_context(tc.tile_pool(name="mm", bufs=2, space="PSUM"))
    ob_pool = ctx.enter_context(tc.tile_pool(name="ob", bufs=3))

    # prefetch all inputs so they never queue behind output DMAs
    in_tiles = []
    fo = bo = 0
    for si, npk in enumerate(NPKS):
        fT = in_pool.tile([128, 256], FP, tag="fc", name=f"fT{si}")
        nc.sync.dma_start(fT[:, :2 * npk * 4], featc[:, fo:fo + 2 * npk * 4])
        btT = in_pool.tile([128, 128], F16, tag="btc", name=f"btT{si}")
        nc.sync.dma_start(btT[:, :npk * 4], btc[:, bo:bo + npk * 4])
        in_tiles.append((fT, btT))
        fo += 2 * npk * 4
        bo += npk * 4

    def build(si):
        npk = NPKS[si]
        fT, btT = in_tiles[si]
        P4 = P_tiles[si % 2][:, :npk * 128].rearrange(
            "p (pt g s) -> p pt g s", g=4, s=32)
        btv = btT[:, :npk * 4].rearrange("p (pt g) -> p pt g", g=4)
        for s in range(6):
            ei, k = divmod(s, 2)
            fv = fT[:, k * npk * 4:(k + 1) * npk * 4].rearrange(
                "p (pt g) -> p pt g", g=4)
            nc.vector.scalar_tensor_tensor(P4[:, :, :, s], btv, float(ei + 1),
                                           fv, op0=eq, op1=mult)
        for ee in range(3):
            nc.vector.tensor_scalar(P4[:, :, :, 6 + ee], btv, float(ee + 1),
                                    None, op0=eq)

    build(0)
    ocol = 0
    for si, npk in enumerate(NPKS):
        P = P_tiles[si % 2]
        built_next = si + 1 >= len(NPKS)
        for blk in range(0, npk, 8):
            bpk = min(8, npk - blk)
            ob = ob_pool.tile([128, 8 * 512], F16)
            for q0 in range(0, bpk, 4):
                gn = min(4, bpk - q0)
                mm = mm_pool.tile([128, 2048], FP)
                for h in range(gn):
                    pt = blk + q0 + h
                    x32 = x_pool.tile([128, 128], F16)
                    nc.vector.transpose(x32[:], P[:, pt * 128:(pt + 1) * 128])
                    nc.tensor.matmul(mm[:, h * 512:(h + 1) * 512], x32[:],
                                     wbig_sb[:], start=True, stop=True)
                nc.scalar.activation(ob[:, q0 * 512:(q0 + gn) * 512],
                                     mm[:, :gn * 512],
                                     mybir.ActivationFunctionType.Tanh)
                if not built_next and blk + q0 >= 4:
                    build(si + 1)
                    built_next = True
            nc.sync.dma_start(
                out[:, ocol + blk * 512:ocol + (blk + bpk) * 512],
                ob[:, :bpk * 512])
        ocol += npk * 512


def build_nc():
    if "nc" in _NC_CACHE:
        return _NC_CACHE["nc"]
    nc = bacc.Bacc("TRN2", target_bir_lowering=False, debug=False)
    fcols = sum(2 * npk * 4 for npk in NPKS)
    bcols = sum(npk * 4 for npk in NPKS)
    ocols = sum(npk * 512 for npk in NPKS)
    featc = nc.dram_tensor("featc", [128, fcols], FP, kind="ExternalInput").ap()
    btc = nc.dram_tensor("btc", [128, bcols], F16, kind="ExternalInput").ap()
    wbig = nc.dram_tensor("wbig", [128, 512], F16, kind="ExternalInput").ap()
    out = nc.dram_tensor("out", [128, ocols], F16, kind="ExternalOutput").ap()
    with tile.TileContext(nc) as tc:
        with ExitStack() as ctx:
            _body(ctx, tc, out, featc, btc, wbig)
    nc.compile()
    _NC_CACHE["nc"] = nc
    return nc


def make_wbig(W_slack, b_slack, W_gen, b_gen, W_load, b_load):
    W_list = [np.asarray(w, np.float32) for w in (W_slack, W_gen, W_load)]
    b_list = [np.asarray(b, np.float32) for b in (b_slack, b_gen, b_load)]
    WBig = np.zeros((128, 512), np.float16)
    for A in range(4):
        col = A * 128
        for s in range(6):
            ei, k = divmod(s, 2)
            WBig[32 * A + s, col:col + 128] = W_list[ei][k].astype(np.float16)
        for ee in range(3):
            WBig[32 * A + 6 + ee, col:col + 128] = b_list[ee].astype(np.float16)
    return WBig


def _permute_inputs(featp, btp):
    """featp (npad, 2) f32, btp (npad,) -> featd (8, 128, 2*984) f32,
    btd (8, 128, 984) f16 in the device layout."""
    featp = featp.reshape(N_CORES, PER_CORE, 2)
    btp = btp.reshape(N_CORES, PER_CORE)
    fparts, bparts = [], []
    off = 0
    for ssz, npk in zip(SUPERS, NPKS):
        f4 = featp[:, off:off + ssz].reshape(N_CORES, 4, 32, npk, 4, 2)
        # (c, B, j, pt, A, k) -> (c, A, j, pt, B, k) -> (c, 128, k, pt*g)
        dev = f4.transpose(0, 4, 2, 3, 1, 5).reshape(N_CORES, 128, npk * 4, 2)
        fparts.append(dev.transpose(0, 1, 3, 2).reshape(N_CORES, 128, -1))
        b4 = btp[:, off:off + ssz].reshape(N_CORES, 4, 32, npk, 4)
        bparts.append(b4.transpose(0, 4, 2, 3, 1).reshape(N_CORES, 128, -1))
        off += ssz
    featd = np.ascontiguousarray(np.concatenate(fparts, axis=2))
    btd = np.ascontiguousarray(
        np.concatenate(bparts, axis=2).astype(np.float16))
    return featd, btd


def kernel(feat, bus_type, W_slack, b_slack, W_gen, b_gen, W_load, b_load,
           **run_kwargs):
    feat = np.asarray(feat, np.float32)
    bt = np.asarray(bus_type)
    n = feat.shape[0]
    npad = N_CORES * PER_CORE
    assert n <= npad

    featp = np.zeros((npad, 2), np.float32)
    featp[:n] = feat
    btp = np.zeros(npad, np.float32)
    btp[:n] = bt.astype(np.float32)
    featd, btd = _permute_inputs(featp, btp)
    wbig = make_wbig(W_slack, b_slack, W_gen, b_gen, W_load, b_load)

    nc = build_nc()
    in_maps = [
        {"featc": featd[i], "btc": btd[i], "wbig": wbig}
        for i in range(N_CORES)
    ]
    try:
        res = run_bass_kernel_spmd(nc, in_maps, list(range(N_CORES)),
                                   **run_kwargs)
    except Exception:
        # A previously-failed process can leave the NeuronCores wedged
        # (NRT_EXEC_UNIT_UNRECOVERABLE); a small probe op resets them.
        import time as _time

        import jax.numpy as jnp

        for _ in range(3):
            try:
                float(jnp.sum(jnp.ones((8, 8))))
                break
            except Exception:
                _time.sleep(5)
        res = run_bass_kernel_spmd(nc, in_maps, list(range(N_CORES)),
                                   **run_kwargs)

    outs = []
    for i in range(N_CORES):
        dev = res.results[i]["out"]  # (128, 125952) f16
        off = 0
        parts = []
        for ssz, npk in zip(SUPERS, NPKS):
            block = dev[:, off:off + npk * 512].reshape(128, npk, 4, 128)
            parts.append(block.reshape(ssz, D))
            off += npk * 512
        outs.append(np.concatenate(parts, axis=0))
    out = np.concatenate(outs, axis=0)
    kernel.last_result = res
    return out[:n].astype(np.float32)


# revision 3
# speedup vs baseline: 1.6467x; 1.0120x over previous
"""V4: fp16-output MoE routing kernel.

The correctness gate is rel_err < 2e-2; plain fp16 arithmetic gives ~1.4e-3,
so no Dekker splits; fp16 output halves HBM write traffic. Engine budget per
core (125,952 tokens): ACT tanh ~116us gapless phase (bottleneck), DMA
~107us, DVE ~60us, PE ~55us. v4 focuses on the startup ramp (22.9us in v3)
and tail.

Data flow per 512-token pack:
- P[p, 32g+s] fp16 slot matrix, 9 slots used per 32-slot group, built by a
  SINGLE scalar_tensor_tensor per supertile: (btE == 1) * fx, where the host
  ships btE = bus_type - e and fx = [f0, f1, 1] triples per expert block so
  the DVE write runs are 9 contiguous elements (s = 3e+j).
- DVE stream-transpose (32x32 blocks): X32[32A+i, 32B+j] = P[32A+j, 32B+i]
- matmul(out, lhsT=X32, rhs=Wbig) with Wbig block-diagonal over A-blocks:
    out[32B+j, A*128+d] = z(token(p=32A+j, g=B), d)    (one PSUM bank/pack)
- ACT tanh over 4 packs (FD=2048, 4 PSUM banks double-buffered), fp16 SBUF
- output dumped linearly [128, npk*512]; the host input permutation is
  chosen so per-partition HBM lines are contiguous and the host-side unpack
  is a pure reshape: orig token q*(npk*4) + pt*4 + A sits at device slot
  (p = 32A + (q%32), pack pt, g = q//32).
"""

import sys
from contextlib import ExitStack

import numpy as np

sys.path.insert(0, "/opt/trn_rl_repo")

import concourse.bacc as bacc  # noqa: E402
import concourse.mybir as mybir  # noqa: E402
import concourse.tile as tile  # noqa: E402
from concourse.bass_utils import run_bass_kernel_spmd  # noqa: E402

FP = mybir.dt.float32
F16 = mybir.dt.float16
D = 128
PACK = 512
SUPERS = [16384] * 7 + [11264]
NPKS = [s // PACK for s in SUPERS]
N_CORES = 8
PER_CORE = sum(SUPERS)

_NC_CACHE = {}


def _body(ctx, tc, out, btc, fxc, wbig):
    nc = tc.nc
    eq = mybir.AluOpType.is_equal
    mult = mybir.AluOpType.mult

    const_pool = ctx.enter_context(tc.tile_pool(name="const", bufs=1))
    wbig_sb = const_pool.tile([128, 512], F16)
    nc.sync.dma_start(wbig_sb[:], wbig)
    # Persistent double-buffered slot matrices; slots 9..31 are zeroed once
    # and never written again (their Wbig rows are zero, but NaN garbage
    # would still poison the accumulation, so the memset is required).
    P_tiles = [const_pool.tile([128, 32 * 128], F16, name=f"Pbuf{i}")
               for i in range(2)]
    # First 8 packs of P0 on DVE (blocks the very first build); the rest on
    # the otherwise-idle GpSimd engine, off the critical path.
    nc.vector.memset(P_tiles[0][:, :8 * 128], 0.0)
    nc.gpsimd.memset(P_tiles[0][:, 8 * 128:], 0.0)
    nc.gpsimd.memset(P_tiles[1][:], 0.0)

    in_pool = ctx.enter_context(tc.tile_pool(name="inp", bufs=8))
    x_pool = ctx.enter_context(tc.tile_pool(name="x32", bufs=8))
    mm_pool = ctx.enter_context(tc.tile_pool(name="mm", bufs=2, space="PSUM"))
    ob_pool = ctx.enter_context(tc.tile_pool(name="ob", bufs=3))

    # prefetch all inputs so they never queue behind output DMAs
    in_tiles = []
    col = 0
    for si, npk in enumerate(NPKS):
        btT = in_pool.tile([128, 32 * 36], F16, tag="btc", name=f"btT{si}")
        nc.sync.dma_start(btT[:, :npk * 36], btc[:, col:col + npk * 36])
        fxT = in_pool.tile([128, 32 * 36], F16, tag="fxc", name=f"fxT{si}")
        nc.sync.dma_start(fxT[:, :npk * 36], fxc[:, col:col + npk * 36])
        in_tiles.append((btT, fxT))
        col += npk * 36

    def build(si, lo, hi):
        btT, fxT = in_tiles[si]
        P4 = P_tiles[si % 2][:, lo * 128:hi * 128].rearrange(
            "p (pt g s) -> p pt g s", g=4, s=32)[:, :, :, 0:9]
        btv = btT[:, lo * 36:hi * 36].rearrange(
            "p (pt g j) -> p pt g j", g=4, j=9)
        fxv = fxT[:, lo * 36:hi * 36].rearrange(
            "p (pt g j) -> p pt g j", g=4, j=9)
        nc.vector.scalar_tensor_tensor(P4, btv, 1.0, fxv, op0=eq, op1=mult)

    build(0, 0, min(8, NPKS[0]))
    ocol = 0
    for si, npk in enumerate(NPKS):
        P = P_tiles[si % 2]
        for blk in range(0, npk, 4):
            gn = min(4, npk - blk)
            ob = ob_pool.tile([128, 2048], F16)
            mm = mm_pool.tile([128, 2048], FP)
            for h in range(gn):
                pt = blk + h
                x32 = x_pool.tile([128, 128], F16)
                nc.vector.transpose(x32[:], P[:, pt * 128:(pt + 1) * 128])
                nc.tensor.matmul(mm[:, h * 512:(h + 1) * 512], x32[:],
                                 wbig_sb[:], start=True, stop=True)
            nc.scalar.activation(ob[:, :gn * 512], mm[:, :gn * 512],
                                 mybir.ActivationFunctionType.Tanh)
            if si == 0 and blk == 0 and npk > 8:
                build(0, 8, npk)
            if si + 1 < len(NPKS) and blk == 4:
                build(si + 1, 0, NPKS[si + 1])
            nc.sync.dma_start(
                out[:, ocol + blk * 512:ocol + (blk + gn) * 512],
                ob[:, :gn * 512])
        ocol += npk * 512


def build_nc():
    if "nc" in _NC_CACHE:
        return _NC_CACHE["nc"]
    nc = bacc.Bacc("TRN2", target_bir_lowering=False, debug=False)
    icols = sum(npk * 36 for npk in NPKS)
    ocols = sum(npk * 512 for npk in NPKS)
    btc = nc.dram_tensor("btc", [128, icols], F16, kind="ExternalInput").ap()
    fxc = nc.dram_tensor("fxc", [128, icols], F16, kind="ExternalInput").ap()
    wbig = nc.dram_tensor("wbig", [128, 512], F16, kind="ExternalInput").ap()
    out = nc.dram_tensor("out", [128, ocols], F16, kind="ExternalOutput").ap()
    with tile.TileContext(nc) as tc:
        with ExitStack() as ctx:
            _body(ctx, tc, out, btc, fxc, wbig)
    nc.compile()
    _NC_CACHE["nc"] = nc
    return nc


def make_wbig(W_slack, b_slack, W_gen, b_gen, W_load, b_load):
    W_list = [np.asarray(w, np.float32) for w in (W_slack, W_gen, W_load)]
    b_list = [np.asarray(b, np.float32) for b in (b_slack, b_gen, b_load)]
    WBig = np.zeros((128, 512), np.float16)
    for A in range(4):
        col = A * 128
        for e in range(3):
            WBig[32 * A + 3 * e + 0, col:col + 128] = \
                W_list[e][0].astype(np.float16)
            WBig[32 * A + 3 * e + 1, col:col + 128] = \
                W_list[e][1].astype(np.float16)
            WBig[32 * A + 3 * e + 2, col:col + 128] = \
                b_list[e].astype(np.float16)
    return WBig


def _permute_inputs(featp, btp):
    """featp (npad, 2) f32, btp (npad,) f32 -> btd, fxd (8, 128, 8856) f16
    in the device layout: per supertile, columns (pt, g, e, j) where
    btd = bus_type - e and fxd = [f0, f1, 1]."""
    featp = featp.reshape(N_CORES, PER_CORE, 2)
    btp = btp.reshape(N_CORES, PER_CORE)
    bparts, fparts = [], []
    off = 0
    erange = np.arange(3, dtype=np.float32)
    for ssz, npk in zip(SUPERS, NPKS):
        f4 = featp[:, off:off + ssz].reshape(N_CORES, 4, 32, npk, 4, 2)
        # orig (c, B, j, pt, A, k) -> device (c, p=32A+j, pt, g=B, k)
        dv = f4.transpose(0, 4, 2, 3, 1, 5).reshape(N_CORES, 128, npk, 4, 2)
        fx3 = np.empty((N_CORES, 128, npk, 4, 3), np.float16)
        fx3[..., :2] = dv
        fx3[..., 2] = 1.0
        fx9 = np.broadcast_to(fx3[:, :, :, :, None, :],
                              (N_CORES, 128, npk, 4, 3, 3))
        fparts.append(fx9.reshape(N_CORES, 128, npk * 36))
        b4 = btp[:, off:off + ssz].reshape(N_CORES, 4, 32, npk, 4)
        db = b4.transpose(0, 4, 2, 3, 1).reshape(N_CORES, 128, npk, 4)
        btE = (db[..., None] - erange)[..., None]
        btE = np.broadcast_to(btE, (N_CORES, 128, npk, 4, 3, 3))
        bparts.append(btE.astype(np.float16).reshape(N_CORES, 128, npk * 36))
        off += ssz
    btd = np.ascontiguousarray(np.concatenate(bparts, axis=2))
    fxd = np.ascontiguousarray(np.concatenate(fparts, axis=2))
    return btd, fxd


def kernel(feat, bus_type, W_slack, b_slack, W_gen, b_gen, W_load, b_load,
           **run_kwargs):
    feat = np.asarray(feat, np.float32)
    bt = np.asarray(bus_type)
    n = feat.shape[0]
    npad = N_CORES * PER_CORE
    assert n <= npad

    featp = np.zeros((npad, 2), np.float32)
    featp[:n] = feat
    btp = np.zeros(npad, np.float32)
    btp[:n] = bt.astype(np.float32)
    btd, fxd = _permute_inputs(featp, btp)
    wbig = make_wbig(W_slack, b_slack, W_gen, b_gen, W_load, b_load)

    nc = build_nc()
    in_maps = [
        {"btc": btd[i], "fxc": fxd[i], "wbig": wbig}
        for i in range(N_CORES)
    ]
    try:
        res = run_bass_kernel_spmd(nc, in_maps, list(range(N_CORES)),
                                   **run_kwargs)
    except Exception:
        # A previously-failed process can leave the NeuronCores wedged
        # (NRT_EXEC_UNIT_UNRECOVERABLE); a small probe op resets them.
        import time as _time

        import jax.numpy as jnp

        for _ in range(3):
            try:
                float(jnp.sum(jnp.ones((8, 8))))
                break
            except Exception:
                _time.sleep(5)
        res = run_bass_kernel_spmd(nc, in_maps, list(range(N_CORES)),
                                   **run_kwargs)

    outs = []
    for i in range(N_CORES):
        dev = res.results[i]["out"]  # (128, 125952) f16
        off = 0
        parts = []
        for ssz, npk in zip(SUPERS, NPKS):
            block = dev[:, off:off + npk * 512].reshape(128, npk, 4, 128)
            parts.append(block.reshape(ssz, D))
            off += npk * 512
        outs.append(np.concatenate(parts, axis=0))
    out = np.concatenate(outs, axis=0)
    kernel.last_result = res
    return out[:n].astype(np.float32)
